# revision 18
# baseline (speedup 1.0000x reference)
"""Trainium2 Bass kernel for nn_Cell_Linear_Attention.

Computation (reference):
  q = (x @ Wq.T + bq)  -> mean over rows -> softmax over feature dim * D^-0.5
  k = (x @ Wk.T + bk)  -> mean over rows -> softmax over seq dim
  v =  x @ Wv.T + bv   (per row)
  attn = einsum('hnd,hmd->hnm', q, k)           [H, C, C]
  ctx  = einsum('hnd,bhne->bhde', k, v)         [R, H, D, D]
  out  = einsum('hnd,bhde->bhne', q, ctx)       [R, C, E]

Key identity: row-mean commutes with the linear projections, so
  q_mean = (x.mean(0)) @ Wq.T + bq.
Each core holds an 8-row shard of x, computes the partial row-SUM on-chip,
and a chunked AllReduce produces the full sum; 1/64 is folded into the
host-passed Wq.T/Wk.T weights.

Sharding: rows (cells) across 8 cores for v/out; attention head i computed
on core i (same program — per-core attn output is gathered into [H, C, C]).

Layout/dtype strategy:
  - Host passes x shard TRANSPOSED ([E, 8*C]) so the contraction dim (E) is
    on partitions; big projections run as float32r matmuls (full PE rate at
    free dim >= 256, ~1e-4 relative accuracy).
  - v is spilled to DRAM as bf16 and re-read for the context phase.
  - Softmaxes skip max-subtraction (|logits| < ~1), exp fused with bias on
    ScalarE; denominators via accum_out (kT), S-matrix PE sums (qT) and
    ones-vector PE sums (k natural).
  - attn/ctx/out matmuls run in bf16 (operands are on-chip softmax outputs).
"""
import numpy as np

import concourse.bacc as bacc
import concourse.bass as bass
import concourse.mybir as mybir
import concourse.tile as tile
from concourse.bass_utils import run_bass_kernel_spmd

N_CORES = 8
R, C, E = 64, 2048, 512
H, D = 8, 64
RL = R // N_CORES          # rows per core (8)
T = RL * C                 # tokens per core (16384)
CC = 4                     # c-chunks per row (of 512)
CW = C // CC               # c-chunk width (512)
F32 = mybir.dt.float32
F32R = mybir.dt.float32r
BF16 = mybir.dt.bfloat16

_CACHE = {}


def build_nc(debug=False):
    nc = bacc.Bacc()
    xT = nc.dram_tensor("xT", [E, T], F32R, kind="ExternalInput")
    wqT = nc.dram_tensor("wqT", [E, E], F32R, kind="ExternalInput")
    wkT = nc.dram_tensor("wkT", [E, E], F32R, kind="ExternalInput")
    wvT = nc.dram_tensor("wvT", [E, E], F32R, kind="ExternalInput")
    bq_c = nc.dram_tensor("bq_c", [E, 1], F32, kind="ExternalInput")
    bk_c = nc.dram_tensor("bk_c", [E, 1], F32, kind="ExternalInput")
    bk_r = nc.dram_tensor("bk_r", [1, E], F32R, kind="ExternalInput")
    bv_r = nc.dram_tensor("bv_r", [1, E], F32R, kind="ExternalInput")
    ones_r = nc.dram_tensor("ones_r", [1, 128], F32R, kind="ExternalInput")
    ones_cb = nc.dram_tensor("ones_cb", [128, 1], BF16, kind="ExternalInput")
    s_sel = nc.dram_tensor("s_sel", [E, H], BF16, kind="ExternalInput")
    s_selT = nc.dram_tensor("s_selT", [H, E], BF16, kind="ExternalInput")
    ones_rb = nc.dram_tensor("ones_rb", [1, 128], BF16, kind="ExternalInput")
    # per-core one-hot selector extracting this core's head from qT/kT
    s_head = nc.dram_tensor("s_head", [E, D], BF16, kind="ExternalInput")
    out = nc.dram_tensor("out", [T, E], F32, kind="ExternalOutput")
    attn = nc.dram_tensor("attn", [C, C], F32, kind="ExternalOutput")
    if debug:
        v_dbg = nc.dram_tensor("v_dbg", [T, E], F32, kind="ExternalOutput")
        kne_dbg = nc.dram_tensor("kne_dbg", [C, E], F32, kind="ExternalOutput")
        zkr_dbg = nc.dram_tensor("zkr_dbg", [1, E], F32, kind="ExternalOutput")
        repk_dbg = nc.dram_tensor("repk_dbg", [128, E], F32, kind="ExternalOutput")
        kn_dbg = nc.dram_tensor("kn_dbg", [C, E], F32, kind="ExternalOutput")
        ctx_dbg = nc.dram_tensor("ctx_dbg", [RL * 128, E], F32,
                                 kind="ExternalOutput")

    with tile.TileContext(nc) as tc:
        sb_ = (lambda n: max(4, n // 2)) if debug else (lambda n: n)
        with (
            tc.tile_pool(name="stream", bufs=sb_(8)) as p_stream,
            tc.tile_pool(name="wts", bufs=1) as p_wts,
            tc.tile_pool(name="psacc", bufs=1) as p_ps,
            tc.tile_pool(name="xbar", bufs=1) as p_xbar,
            tc.tile_pool(name="qk", bufs=1) as p_qk,
            tc.tile_pool(name="vout", bufs=sb_(4)) as p_vout,
            tc.tile_pool(name="vload", bufs=sb_(6)) as p_vload,
            tc.tile_pool(name="bd", bufs=sb_(8)) as p_bd,
            tc.tile_pool(name="ocp", bufs=sb_(3)) as p_ocp,
            tc.tile_pool(name="acp", bufs=sb_(3)) as p_acp,
            tc.tile_pool(name="zb", bufs=1) as p_zb,
            tc.tile_pool(name="mm", bufs=3, space="PSUM") as p_mm,
            tc.tile_pool(name="ctxp", bufs=2, space="PSUM") as p_ctx,
            tc.tile_pool(name="zp", bufs=1, space="PSUM") as p_z,
            tc.tile_pool(name="dram", bufs=1, space="DRAM") as p_dram,
        ):
            # ---- persistent SBUF tensors ----
            wq = [p_wts.tile([128, E], F32R, name=f"wq{t}") for t in range(4)]
            wk = [p_wts.tile([128, E], F32R, name=f"wk{t}") for t in range(4)]
            wv = [p_wts.tile([128, E], F32R, name=f"wv{t}") for t in range(4)]
            for t in range(4):
                nc.sync.dma_start(out=wq[t], in_=wqT[128 * t:128 * (t + 1), :])
                nc.sync.dma_start(out=wk[t], in_=wkT[128 * t:128 * (t + 1), :])
                nc.sync.dma_start(out=wv[t], in_=wvT[128 * t:128 * (t + 1), :])
            bqc = [p_wts.tile([128, 1], F32, name=f"bqc{t}") for t in range(4)]
            bkc = [p_wts.tile([128, 1], F32, name=f"bkc{t}") for t in range(4)]
            for t in range(4):
                nc.sync.dma_start(out=bqc[t], in_=bq_c[128 * t:128 * (t + 1), :])
                nc.sync.dma_start(out=bkc[t], in_=bk_c[128 * t:128 * (t + 1), :])
            bkr = p_wts.tile([1, E], F32R)
            nc.sync.dma_start(out=bkr, in_=bk_r[:, :])
            bvr = p_wts.tile([1, E], F32R)
            nc.sync.dma_start(out=bvr, in_=bv_r[:, :])
            onesr = p_wts.tile([1, 128], F32R)
            nc.sync.dma_start(out=onesr, in_=ones_r[:, :])
            onescb = p_wts.tile([128, 1], BF16)
            nc.sync.dma_start(out=onescb, in_=ones_cb[:, :])
            sselT = p_wts.tile([H, E], BF16)
            nc.sync.dma_start(out=sselT, in_=s_selT[:, :])
            onesrb = p_wts.tile([1, 128], BF16)
            nc.sync.dma_start(out=onesrb, in_=ones_rb[:, :])
            ssel = [p_wts.tile([128, H], BF16, name=f"ssel{t}") for t in range(4)]
            shead = [p_wts.tile([128, D], BF16, name=f"shead{t}") for t in range(4)]
            for t in range(4):
                nc.sync.dma_start(out=ssel[t], in_=s_sel[128 * t:128 * (t + 1), :])
                nc.sync.dma_start(out=shead[t], in_=s_head[128 * t:128 * (t + 1), :])

            ps = [p_ps.tile([128, C], F32R, name=f"ps{t}") for t in range(4)]
            v_sp = p_dram.tile([T, E], BF16)
            # contiguous per-chunk AllReduce buffers (one Shared out each —
            # a Shared DRAM tensor may only have a single writing inst)
            ar_in = p_dram.tile([CC, E, CW], F32R)
            ar_outs = [p_dram.tile([E, CW], F32R, addr_space="Shared",
                                   name=f"ar_out{i}") for i in range(CC)]

            # ============ P1: stream xT, partial row-sum + v projection ====
            x4 = xT.rearrange("(t p) (b c) -> t p b c", p=128, c=C)
            v3 = v_sp.rearrange("(n p) e -> n p e", p=128)
            for cc in range(CC):
                for b in range(RL):
                    xt = [p_stream.tile([128, CW], F32R, name="xt", tag="xt")
                          for _ in range(4)]
                    for t in range(4):
                        nc.sync.dma_start(
                            out=xt[t], in_=x4[t, :, b, cc * CW:(cc + 1) * CW])
                    # partial row-sum (DVE, on f32 view of the f32r bits)
                    for t in range(4):
                        dst = ps[t][:, cc * CW:(cc + 1) * CW].bitcast(F32)
                        src = xt[t].bitcast(F32)
                        if b == 0:
                            nc.vector.tensor_copy(dst, src)
                        else:
                            nc.vector.tensor_add(dst, dst, src)
                    # v projection for these 4 token-subtiles
                    for m in range(4):
                        vps = p_mm.tile([128, E], F32, name="vps", tag="mm")
                        for t in range(4):
                            nc.tensor.matmul(
                                vps, xt[t][:, m * 128:(m + 1) * 128], wv[t],
                                start=(t == 0), stop=False)
                        nc.tensor.matmul(vps, onesr, bvr, start=False, stop=True)
                        vsb = p_vout.tile([128, E], BF16, name="vsb")
                        nc.scalar.activation(
                            vsb, vps, mybir.ActivationFunctionType.Copy)
                        nidx = b * 16 + cc * 4 + m
                        nc.sync.dma_start(out=v3[nidx], in_=vsb)
                        if debug and False:
                            vdb = p_vout.tile([128, E], F32, name="vdb")
                            nc.vector.tensor_copy(vdb, vsb)
                            nc.sync.dma_start(
                                out=v_dbg.rearrange("(n p) e -> n p e", p=128)[nidx],
                                in_=vdb)
                # ps chunk complete -> stage for AllReduce
                for t in range(4):
                    nc.sync.dma_start(
                        out=ar_in[cc, 128 * t:128 * (t + 1), :],
                        in_=ps[t][:, cc * CW:(cc + 1) * CW])
                nc.gpsimd.collective_compute(
                    "AllReduce", mybir.AluOpType.add,
                    ins=[ar_in[cc]],
                    outs=[ar_outs[cc][:, :]],
                    replica_groups=[list(range(N_CORES))],
                )

            # ============ P2: projections of the row-mean =================
            # xbar rows (x SUM; 1/64 folded into wq/wk on host)
            xb = [p_xbar.tile([128, C], F32R, name=f"xb{t}") for t in range(4)]
            qt_e = [p_qk.tile([128, C], BF16, name=f"qte{t}") for t in range(4)]
            kt_e = [p_qk.tile([128, C], BF16, name=f"kte{t}") for t in range(4)]
            kn_e = [p_qk.tile([128, E], BF16, name=f"kne{i}") for i in range(16)]
            zk_parts = [p_qk.tile([128, CC], F32, name=f"zkp{t}") for t in range(4)]
            for cc in range(CC):
                cs = slice(cc * CW, (cc + 1) * CW)
                for t in range(4):
                    nc.sync.dma_start(out=xb[t][:, cs],
                                      in_=ar_outs[cc][128 * t:128 * (t + 1), :])
                # qT / kT projections: [feat 128, c 512] tiles
                for ft in range(4):
                    qps = p_mm.tile([128, CW], F32, name="qps", tag="mm")
                    for t in range(4):
                        nc.tensor.matmul(qps, wq[t][:, ft * 128:(ft + 1) * 128],
                                         xb[t][:, cs], start=(t == 0),
                                         stop=(t == 3))
                    nc.scalar.activation(qt_e[ft][:, cs], qps,
                                         mybir.ActivationFunctionType.Exp,
                                         bias=bqc[ft])
                    kps = p_mm.tile([128, CW], F32, name="kps", tag="mm")
                    for t in range(4):
                        nc.tensor.matmul(kps, wk[t][:, ft * 128:(ft + 1) * 128],
                                         xb[t][:, cs], start=(t == 0),
                                         stop=(t == 3))
                    nc.scalar.activation(kt_e[ft][:, cs], kps,
                                         mybir.ActivationFunctionType.Exp,
                                         bias=bkc[ft],
                                         accum_out=zk_parts[ft][:, cc:cc + 1])
                # k natural projection: [c 128, feat 512] tiles
                for m in range(4):
                    ci = cc * 4 + m
                    kns = p_mm.tile([128, E], F32, name="kns", tag="mm")
                    for t in range(4):
                        nc.tensor.matmul(
                            kns, xb[t][:, cc * CW + m * 128:cc * CW + (m + 1) * 128],
                            wk[t], start=(t == 0), stop=False)
                    nc.tensor.matmul(kns, onesr, bkr, start=False, stop=True)
                    nc.scalar.activation(kn_e[ci], kns,
                                         mybir.ActivationFunctionType.Exp)

            # ============ P3: softmax denominators ========================
            # Zq[h, c] via S-matrix PE sums over feature groups
            invzq = [p_zb.tile([H, CW], F32, name=f"invzq{cc}") for cc in range(CC)]
            for cc in range(CC):
                cs = slice(cc * CW, (cc + 1) * CW)
                zq = p_z.tile([H, CW], F32, name="zq", tag="zq")
                for t in range(4):
                    nc.tensor.matmul(zq, ssel[t], qt_e[t][:, cs],
                                     start=(t == 0), stop=(t == 3))
                nc.vector.reciprocal(invzq[cc], zq)
                nc.vector.tensor_scalar_mul(invzq[cc], invzq[cc], float(D) ** -0.5)
            # qT_soft: replicate invzq rows across feature partitions
            # via a tiny PE matmul (sselT.T @ invzq), then elementwise mul.
            invzq_bf = p_zb.tile([H, C], BF16)
            for cc in range(CC):
                nc.vector.tensor_copy(invzq_bf[:, cc * CW:(cc + 1) * CW],
                                      invzq[cc])
            for ft in range(4):
                for cc in range(CC):
                    cs = slice(cc * CW, (cc + 1) * CW)
                    rep = p_mm.tile([128, CW], F32, name="rep", tag="mm")
                    nc.tensor.matmul(rep, sselT[:, ft * 128:(ft + 1) * 128],
                                     invzq_bf[:, cs], start=True, stop=True)
                    nc.vector.tensor_mul(qt_e[ft][:, cs], qt_e[ft][:, cs], rep)
            # kT_soft: per-feature 1/sum over all c
            for ft in range(4):
                zsum = p_qk.tile([128, 1], F32, name="zsum", tag="zsum")
                nc.vector.tensor_reduce(zsum, zk_parts[ft],
                                        axis=mybir.AxisListType.X,
                                        op=mybir.AluOpType.add)
                nc.vector.reciprocal(zsum, zsum)
                nc.vector.tensor_scalar_mul(kt_e[ft], kt_e[ft], zsum)
            # k_nat_soft: Zk row via ones-vector PE sums (bf16 path)
            zkr = p_z.tile([1, E], F32, name="zkr", tag="zkr")
            for ci in range(16):
                nc.tensor.matmul(zkr, onescb, kn_e[ci],
                                 start=(ci == 0), stop=(ci == 15))
            if debug:
                zkd = p_zb.tile([1, E], F32)
                nc.vector.tensor_copy(zkd, zkr)
                nc.sync.dma_start(out=zkr_dbg[:, :], in_=zkd)
                for ci in range(16):
                    kned = p_ocp.tile([128, E], F32, name="kned", bufs=1)
                    nc.vector.tensor_copy(kned, kn_e[ci])
                    nc.sync.dma_start(
                        out=kne_dbg.rearrange("(n p) e -> n p e", p=128)[ci],
                        in_=kned)
            invzkr = p_zb.tile([1, E], F32)
            nc.vector.reciprocal(invzkr, zkr)
            invzkr_bf = p_zb.tile([1, E], BF16)
            nc.vector.tensor_copy(invzkr_bf, invzkr)
            repk = p_mm.tile([128, E], F32, name="repk", tag="mm")
            nc.tensor.matmul(repk, onesrb, invzkr_bf, start=True, stop=True)
            repk_sb = p_qk.tile([128, E], BF16)
            nc.vector.tensor_copy(repk_sb, repk)
            if debug:
                rkd = p_ocp.tile([128, E], F32, name="rkd", bufs=1)
                nc.vector.tensor_copy(rkd, repk_sb)
                nc.sync.dma_start(out=repk_dbg[:, :], in_=rkd)
            for ci in range(16):
                nc.vector.tensor_mul(kn_e[ci], kn_e[ci], repk_sb)
            if debug and False:
                for ci in range(16):
                    knd = p_ocp.tile([128, E], F32, name="knd")
                    nc.vector.tensor_copy(knd, kn_e[ci])
                    nc.sync.dma_start(
                        out=kn_dbg.rearrange("(n p) e -> n p e", p=128)[ci],
                        in_=knd)

            # ============ P4: attention map for this core's head ==========
            # Extract this core's head rows from qT_soft/kT_soft with the
            # per-core one-hot selector (exact in bf16), then attn = hq.T@hk.
            hq = p_qk.tile([D, C], BF16)
            hk = p_qk.tile([D, C], BF16)
            for cc in range(CC):
                cs = slice(cc * CW, (cc + 1) * CW)
                hqp = p_mm.tile([D, CW], F32, name="hqp", tag="mm")
                for t in range(4):
                    nc.tensor.matmul(hqp, shead[t], qt_e[t][:, cs],
                                     start=(t == 0), stop=(t == 3))
                nc.vector.tensor_copy(hq[:, cs], hqp)
                hkp = p_mm.tile([D, CW], F32, name="hkp", tag="mm")
                for t in range(4):
                    nc.tensor.matmul(hkp, shead[t], kt_e[t][:, cs],
                                     start=(t == 0), stop=(t == 3))
                nc.vector.tensor_copy(hk[:, cs], hkp)
            for nt in range(16):
                for ms in range(CC):
                    aps_ = p_mm.tile([128, CW], F32, name="aps", tag="mm")
                    nc.tensor.matmul(aps_, hq[:, nt * 128:(nt + 1) * 128],
                                     hk[:, ms * CW:(ms + 1) * CW],
                                     start=True, stop=True)
                    asb = p_acp.tile([128, CW], F32, name="asb")
                    nc.scalar.activation(asb, aps_,
                                         mybir.ActivationFunctionType.Copy)
                    nc.sync.dma_start(
                        out=attn[nt * 128:(nt + 1) * 128,
                                 ms * CW:(ms + 1) * CW],
                        in_=asb)

            # ============ P5+P6: context + output =========================
            v3r = v_sp.rearrange("(n p) e -> n p e", p=128)
            o3 = out.rearrange("(n p) e -> n p e", p=128)
            for b in range(RL):
                ctxp = p_ctx.tile([128, E], F32, name="ctxp", tag="ctx")
                for nk in range(16):
                    vsb = p_vload.tile([128, E], BF16, name="vld", tag="vld")
                    nc.sync.dma_start(out=vsb, in_=v3r[b * 16 + nk])
                    # NOTE: start=True clears has_written BITS for the whole
                    # bank, so only the first matmul of the bank may set it
                    # (other quarters then overwrite-on-first-touch since
                    # their bits are still clear).
                    for hp in range(4):
                        nc.tensor.matmul(
                            ctxp[:, hp * 128:(hp + 1) * 128],
                            kn_e[nk][:, hp * 128:(hp + 1) * 128],
                            vsb[:, hp * 128:(hp + 1) * 128],
                            start=(nk == 0 and hp == 0),
                            stop=(nk == 15 and hp == 3))
                if debug:
                    cdb = p_ocp.tile([128, E], F32, name="cdb", bufs=1)
                    nc.vector.tensor_copy(cdb, ctxp)
                    nc.sync.dma_start(
                        out=ctx_dbg.rearrange("(n p) e -> n p e", p=128)[b],
                        in_=cdb)
                # build block-diag ctx tiles (bf16)
                bd = [p_bd.tile([128, 128], BF16, name="bd", tag="bd")
                      for _ in range(4)]
                for hp in range(4):
                    nc.vector.memset(bd[hp], 0.0)
                    nc.vector.tensor_copy(
                        bd[hp][0:64, 0:64],
                        ctxp[0:64, hp * 128:hp * 128 + 64])
                    nc.vector.tensor_copy(
                        bd[hp][64:128, 64:128],
                        ctxp[64:128, hp * 128 + 64:(hp + 1) * 128])
                for nt in range(16):
                    ops_ = p_mm.tile([128, E], F32, name="ops", tag="mm")
                    for hp in range(4):
                        nc.tensor.matmul(
                            ops_[:, hp * 128:(hp + 1) * 128],
                            qt_e[hp][:, nt * 128:(nt + 1) * 128],
                            bd[hp], start=True, stop=True)
                    osb = p_ocp.tile([128, E], F32, name="osb")
                    nc.vector.tensor_copy(osb, ops_)
                    nc.sync.dma_start(out=o3[b * 16 + nt], in_=osb)

    nc.finalize()
    return nc


def _host_prep(x, Wq, bq, Wk, bk, Wv, bv):
    """Build the 8 per-core input maps."""
    import ml_dtypes
    bf16 = ml_dtypes.bfloat16
    x = np.ascontiguousarray(x, dtype=np.float32)
    wqT = np.ascontiguousarray(Wq.T / R, dtype=np.float32)
    wkT = np.ascontiguousarray(Wk.T / R, dtype=np.float32)
    wvT = np.ascontiguousarray(Wv.T, dtype=np.float32)
    ones_r = np.ones((1, 128), dtype=np.float32)
    ones_cb = np.ones((128, 1), dtype=bf16)
    s_sel = np.zeros((E, H), dtype=bf16)
    for h in range(H):
        s_sel[h * D:(h + 1) * D, h] = 1.0
    s_selT = np.ascontiguousarray(s_sel.T)
    ones_rb = np.ones((1, 128), dtype=bf16)
    in_maps = []
    for c in range(N_CORES):
        xs = x[RL * c:RL * (c + 1)]              # [8, 2048, 512]
        xT = np.ascontiguousarray(xs.transpose(2, 0, 1).reshape(E, T))
        s_head = np.zeros((E, D), dtype=bf16)
        s_head[c * D:(c + 1) * D, :] = np.eye(D, dtype=bf16)
        in_maps.append({
            "xT": xT,
            "wqT": wqT, "wkT": wkT, "wvT": wvT,
            "bq_c": bq.reshape(E, 1).astype(np.float32),
            "bk_c": bk.reshape(E, 1).astype(np.float32),
            "bk_r": bk.reshape(1, E).astype(np.float32),
            "bv_r": bv.reshape(1, E).astype(np.float32),
            "ones_r": ones_r,
            "ones_cb": ones_cb,
            "s_sel": s_sel,
            "s_selT": s_selT,
            "ones_rb": ones_rb,
            "s_head": s_head,
        })
    return in_maps


def kernel(x, Wq, bq, Wk, bk, Wv, bv):
    if "nc" not in _CACHE:
        _CACHE["nc"] = build_nc()
    nc = _CACHE["nc"]
    in_maps = _host_prep(np.asarray(x), np.asarray(Wq), np.asarray(bq),
                         np.asarray(Wk), np.asarray(bk),
                         np.asarray(Wv), np.asarray(bv))
    res = run_bass_kernel_spmd(nc, in_maps, list(range(N_CORES)))
    out = np.concatenate(
        [res.results[c]["out"].reshape(RL, C, E) for c in range(N_CORES)], axis=0)
    attn = np.stack([res.results[c]["attn"] for c in range(N_CORES)], axis=0)
    return out, attn


# revision 26
# speedup vs baseline: 1.1227x; 1.1227x over previous
"""Trainium2 Bass kernel for nn_Cell_Linear_Attention.

Computation (reference):
  q = (x @ Wq.T + bq)  -> mean over rows -> softmax over feature dim * D^-0.5
  k = (x @ Wk.T + bk)  -> mean over rows -> softmax over seq dim
  v =  x @ Wv.T + bv   (per row)
  attn = einsum('hnd,hmd->hnm', q, k)           [H, C, C]
  ctx  = einsum('hnd,bhne->bhde', k, v)         [R, H, D, D]
  out  = einsum('hnd,bhde->bhne', q, ctx)       [R, C, E]

Key identity: row-mean commutes with the linear projections, so
  q_mean = (x.mean(0)) @ Wq.T + bq.
Each core holds an 8-row shard of x, computes the partial row-SUM on-chip,
and a chunked AllReduce produces the full sum; 1/64 is folded into the
host-passed Wq.T/Wk.T weights.

Sharding: rows (cells) across 8 cores for v/out; attention head i computed
on core i (same program — per-core attn output is gathered into [H, C, C]).

Layout/dtype strategy:
  - Host passes x shard TRANSPOSED ([E, 8*C]) so the contraction dim (E) is
    on partitions; big projections run as float32r matmuls (full PE rate at
    free dim >= 256, ~1e-4 relative accuracy).
  - v is spilled to DRAM as bf16 and re-read for the context phase.
  - Softmaxes skip max-subtraction (|logits| < ~1), exp fused with bias on
    ScalarE; denominators via accum_out (kT), S-matrix PE sums (qT) and
    ones-vector PE sums (k natural).
  - attn/ctx/out matmuls run in bf16 (operands are on-chip softmax outputs).
"""
import numpy as np

import concourse.bacc as bacc
import concourse.bass as bass
import concourse.mybir as mybir
import concourse.tile as tile
from concourse.bass_utils import run_bass_kernel_spmd

N_CORES = 8
R, C, E = 64, 2048, 512
H, D = 8, 64
RL = R // N_CORES          # rows per core (8)
T = RL * C                 # tokens per core (16384)
CC = 4                     # c-chunks per row (of 512)
CW = C // CC               # c-chunk width (512)
F32 = mybir.dt.float32
F32R = mybir.dt.float32r
BF16 = mybir.dt.bfloat16

_CACHE = {}


def build_nc(debug=False):
    nc = bacc.Bacc()
    xT = nc.dram_tensor("xT", [E, T], F32R, kind="ExternalInput")
    wqT = nc.dram_tensor("wqT", [E, E], F32R, kind="ExternalInput")
    wkT = nc.dram_tensor("wkT", [E, E], F32R, kind="ExternalInput")
    wvT = nc.dram_tensor("wvT", [E, E], F32R, kind="ExternalInput")
    bq_c = nc.dram_tensor("bq_c", [E, 1], F32, kind="ExternalInput")
    bk_c = nc.dram_tensor("bk_c", [E, 1], F32, kind="ExternalInput")
    bk_r = nc.dram_tensor("bk_r", [1, E], F32R, kind="ExternalInput")
    bv_r = nc.dram_tensor("bv_r", [1, E], F32R, kind="ExternalInput")
    ones_r = nc.dram_tensor("ones_r", [1, 128], F32R, kind="ExternalInput")
    ones_cb = nc.dram_tensor("ones_cb", [128, 1], BF16, kind="ExternalInput")
    s_sel = nc.dram_tensor("s_sel", [E, H], BF16, kind="ExternalInput")
    s_selT = nc.dram_tensor("s_selT", [H, E], BF16, kind="ExternalInput")
    ones_rb = nc.dram_tensor("ones_rb", [1, 128], BF16, kind="ExternalInput")
    # per-core one-hot selector extracting this core's head from qT/kT
    s_head = nc.dram_tensor("s_head", [E, D], BF16, kind="ExternalInput")
    out = nc.dram_tensor("out", [T, E], F32, kind="ExternalOutput")
    attn = nc.dram_tensor("attn", [C, C], F32, kind="ExternalOutput")
    if debug:
        v_dbg = nc.dram_tensor("v_dbg", [T, E], F32, kind="ExternalOutput")
        kne_dbg = nc.dram_tensor("kne_dbg", [C, E], F32, kind="ExternalOutput")
        zkr_dbg = nc.dram_tensor("zkr_dbg", [1, E], F32, kind="ExternalOutput")
        repk_dbg = nc.dram_tensor("repk_dbg", [128, E], F32, kind="ExternalOutput")
        kn_dbg = nc.dram_tensor("kn_dbg", [C, E], F32, kind="ExternalOutput")
        ctx_dbg = nc.dram_tensor("ctx_dbg", [RL * 128, E], F32,
                                 kind="ExternalOutput")

    with tile.TileContext(nc) as tc:
        sb_ = (lambda n: max(4, n // 2)) if debug else (lambda n: n)
        with (
            tc.tile_pool(name="stream", bufs=sb_(8)) as p_stream,
            tc.tile_pool(name="wts", bufs=1) as p_wts,
            tc.tile_pool(name="psacc", bufs=1) as p_ps,
            tc.tile_pool(name="xbar", bufs=1) as p_xbar,
            tc.tile_pool(name="qk", bufs=1) as p_qk,
            tc.tile_pool(name="vout", bufs=sb_(4)) as p_vout,
            tc.tile_pool(name="vload", bufs=sb_(6)) as p_vload,
            tc.tile_pool(name="bd", bufs=sb_(8)) as p_bd,
            tc.tile_pool(name="ocp", bufs=sb_(3)) as p_ocp,
            tc.tile_pool(name="acp", bufs=sb_(3)) as p_acp,
            tc.tile_pool(name="zb", bufs=1) as p_zb,
            tc.tile_pool(name="mm", bufs=4, space="PSUM") as p_mm,
            tc.tile_pool(name="ctxp", bufs=2, space="PSUM") as p_ctx,
            tc.tile_pool(name="zp", bufs=1, space="PSUM") as p_z,
            tc.tile_pool(name="dram", bufs=1, space="DRAM") as p_dram,
        ):
            # ---- persistent SBUF tensors ----
            wq = [p_wts.tile([128, E], F32R, name=f"wq{t}") for t in range(4)]
            wk = [p_wts.tile([128, E], F32R, name=f"wk{t}") for t in range(4)]
            wv = [p_wts.tile([128, E], F32R, name=f"wv{t}") for t in range(4)]
            for t in range(4):
                nc.sync.dma_start(out=wq[t], in_=wqT[128 * t:128 * (t + 1), :])
                nc.sync.dma_start(out=wk[t], in_=wkT[128 * t:128 * (t + 1), :])
                nc.sync.dma_start(out=wv[t], in_=wvT[128 * t:128 * (t + 1), :])
            bqc = [p_wts.tile([128, 1], F32, name=f"bqc{t}") for t in range(4)]
            bkc = [p_wts.tile([128, 1], F32, name=f"bkc{t}") for t in range(4)]
            for t in range(4):
                nc.sync.dma_start(out=bqc[t], in_=bq_c[128 * t:128 * (t + 1), :])
                nc.sync.dma_start(out=bkc[t], in_=bk_c[128 * t:128 * (t + 1), :])
            bkr = p_wts.tile([1, E], F32R)
            nc.sync.dma_start(out=bkr, in_=bk_r[:, :])
            bvr = p_wts.tile([1, E], F32R)
            nc.sync.dma_start(out=bvr, in_=bv_r[:, :])
            onesr = p_wts.tile([1, 128], F32R)
            nc.sync.dma_start(out=onesr, in_=ones_r[:, :])
            onescb = p_wts.tile([128, 1], BF16)
            nc.sync.dma_start(out=onescb, in_=ones_cb[:, :])
            sselT = p_wts.tile([H, E], BF16)
            nc.sync.dma_start(out=sselT, in_=s_selT[:, :])
            onesrb = p_wts.tile([1, 128], BF16)
            nc.sync.dma_start(out=onesrb, in_=ones_rb[:, :])
            ssel = [p_wts.tile([128, H], BF16, name=f"ssel{t}") for t in range(4)]
            shead = [p_wts.tile([128, D], BF16, name=f"shead{t}") for t in range(4)]
            for t in range(4):
                nc.sync.dma_start(out=ssel[t], in_=s_sel[128 * t:128 * (t + 1), :])
                nc.sync.dma_start(out=shead[t], in_=s_head[128 * t:128 * (t + 1), :])

            # bv replicated across partitions once (PE), used as DVE add
            bv_ps = p_mm.tile([128, E], F32, name="bv_ps", tag="mm")
            nc.tensor.matmul(bv_ps, onesr, bvr, start=True, stop=True)
            bv_rep = p_wts.tile([128, E], BF16)
            nc.vector.tensor_copy(bv_rep, bv_ps)

            ps = [p_ps.tile([128, C], F32R, name=f"ps{t}") for t in range(4)]
            v_sp = p_dram.tile([T, E], BF16)
            # contiguous per-chunk AllReduce buffers (one Shared out each —
            # a Shared DRAM tensor may only have a single writing inst)
            ar_in = p_dram.tile([CC, E, CW], F32R)
            ar_outs = [p_dram.tile([E, CW], F32R, addr_space="Shared",
                                   name=f"ar_out{i}") for i in range(CC)]

            # ============ P1: stream xT, partial row-sum + v projection ====
            x4 = xT.rearrange("(t p) (b c) -> t p b c", p=128, c=C)
            v3 = v_sp.rearrange("(n p) e -> n p e", p=128)
            for cc in range(CC):
                for b in range(RL):
                    xt = [p_stream.tile([128, CW], F32R, name="xt", tag="xt")
                          for _ in range(4)]
                    for t in range(4):
                        nc.sync.dma_start(
                            out=xt[t], in_=x4[t, :, b, cc * CW:(cc + 1) * CW])
                    # partial row-sum (DVE, on f32 view of the f32r bits)
                    for t in range(4):
                        dst = ps[t][:, cc * CW:(cc + 1) * CW].bitcast(F32)
                        src = xt[t].bitcast(F32)
                        if b == 0:
                            nc.vector.tensor_copy(dst, src)
                        else:
                            nc.vector.tensor_add(dst, dst, src)
                    # v projection for these 4 token-subtiles
                    for m in range(4):
                        vps = p_mm.tile([128, E], F32, name="vps", tag="mm")
                        for t in range(4):
                            nc.tensor.matmul(
                                vps, xt[t][:, m * 128:(m + 1) * 128], wv[t],
                                start=(t == 0), stop=(t == 3))
                        vsb = p_vout.tile([128, E], BF16, name="vsb")
                        nc.vector.tensor_add(vsb, vps, bv_rep)
                        nidx = b * 16 + cc * 4 + m
                        nc.sync.dma_start(out=v3[nidx], in_=vsb)
                        if debug and False:
                            vdb = p_vout.tile([128, E], F32, name="vdb")
                            nc.vector.tensor_copy(vdb, vsb)
                            nc.sync.dma_start(
                                out=v_dbg.rearrange("(n p) e -> n p e", p=128)[nidx],
                                in_=vdb)
                # ps chunk complete -> stage for AllReduce
                for t in range(4):
                    nc.sync.dma_start(
                        out=ar_in[cc, 128 * t:128 * (t + 1), :],
                        in_=ps[t][:, cc * CW:(cc + 1) * CW])
                nc.gpsimd.collective_compute(
                    "AllReduce", mybir.AluOpType.add,
                    ins=[ar_in[cc]],
                    outs=[ar_outs[cc][:, :]],
                    replica_groups=[list(range(N_CORES))],
                )

            # ============ P2: projections of the row-mean =================
            # xbar rows (x SUM; 1/64 folded into wq/wk on host)
            xb = [p_xbar.tile([128, C], F32R, name=f"xb{t}") for t in range(4)]
            qt_e = [p_qk.tile([128, C], BF16, name=f"qte{t}") for t in range(4)]
            kt_e = [p_qk.tile([128, C], BF16, name=f"kte{t}") for t in range(4)]
            kn_e = [p_qk.tile([128, E], BF16, name=f"kne{i}") for i in range(16)]
            zk_parts = [p_qk.tile([128, CC], F32, name=f"zkp{t}") for t in range(4)]
            for cc in range(CC):
                cs = slice(cc * CW, (cc + 1) * CW)
                for t in range(4):
                    nc.sync.dma_start(out=xb[t][:, cs],
                                      in_=ar_outs[cc][128 * t:128 * (t + 1), :])
                # qT / kT projections: [feat 128, c 512] tiles
                for ft in range(4):
                    qps = p_mm.tile([128, CW], F32, name="qps", tag="mm")
                    for t in range(4):
                        nc.tensor.matmul(qps, wq[t][:, ft * 128:(ft + 1) * 128],
                                         xb[t][:, cs], start=(t == 0),
                                         stop=(t == 3))
                    nc.scalar.activation(qt_e[ft][:, cs], qps,
                                         mybir.ActivationFunctionType.Exp,
                                         bias=bqc[ft])
                    kps = p_mm.tile([128, CW], F32, name="kps", tag="mm")
                    for t in range(4):
                        nc.tensor.matmul(kps, wk[t][:, ft * 128:(ft + 1) * 128],
                                         xb[t][:, cs], start=(t == 0),
                                         stop=(t == 3))
                    nc.scalar.activation(kt_e[ft][:, cs], kps,
                                         mybir.ActivationFunctionType.Exp,
                                         bias=bkc[ft],
                                         accum_out=zk_parts[ft][:, cc:cc + 1])
                # k natural projection: [c 128, feat 512] tiles
                for m in range(4):
                    ci = cc * 4 + m
                    kns = p_mm.tile([128, E], F32, name="kns", tag="mm")
                    for t in range(4):
                        nc.tensor.matmul(
                            kns, xb[t][:, cc * CW + m * 128:cc * CW + (m + 1) * 128],
                            wk[t], start=(t == 0), stop=False)
                    nc.tensor.matmul(kns, onesr, bkr, start=False, stop=True)
                    nc.scalar.activation(kn_e[ci], kns,
                                         mybir.ActivationFunctionType.Exp)

            # ============ P3: softmax denominators ========================
            # Zq[h, c] via S-matrix PE sums over feature groups
            invzq = [p_zb.tile([H, CW], F32, name=f"invzq{cc}") for cc in range(CC)]
            for cc in range(CC):
                cs = slice(cc * CW, (cc + 1) * CW)
                zq = p_z.tile([H, CW], F32, name="zq", tag="zq")
                for t in range(4):
                    nc.tensor.matmul(zq, ssel[t], qt_e[t][:, cs],
                                     start=(t == 0), stop=(t == 3))
                nc.vector.reciprocal(invzq[cc], zq)
                nc.vector.tensor_scalar_mul(invzq[cc], invzq[cc], float(D) ** -0.5)
            # qT_soft: replicate invzq rows across feature partitions
            # via a tiny PE matmul (sselT.T @ invzq), then elementwise mul.
            invzq_bf = p_zb.tile([H, C], BF16)
            for cc in range(CC):
                nc.vector.tensor_copy(invzq_bf[:, cc * CW:(cc + 1) * CW],
                                      invzq[cc])
            for ft in range(4):
                for cc in range(CC):
                    cs = slice(cc * CW, (cc + 1) * CW)
                    rep = p_mm.tile([128, CW], F32, name="rep", tag="mm")
                    nc.tensor.matmul(rep, sselT[:, ft * 128:(ft + 1) * 128],
                                     invzq_bf[:, cs], start=True, stop=True)
                    nc.vector.tensor_mul(qt_e[ft][:, cs], qt_e[ft][:, cs], rep)
            # kT_soft: per-feature 1/sum over all c
            for ft in range(4):
                zsum = p_qk.tile([128, 1], F32, name="zsum", tag="zsum")
                nc.vector.tensor_reduce(zsum, zk_parts[ft],
                                        axis=mybir.AxisListType.X,
                                        op=mybir.AluOpType.add)
                nc.vector.reciprocal(zsum, zsum)
                nc.vector.tensor_scalar_mul(kt_e[ft], kt_e[ft], zsum)
            # k_nat_soft: Zk row via ones-vector PE sums (bf16 path)
            zkr = p_z.tile([1, E], F32, name="zkr", tag="zkr")
            for ci in range(16):
                nc.tensor.matmul(zkr, onescb, kn_e[ci],
                                 start=(ci == 0), stop=(ci == 15))
            if debug:
                zkd = p_zb.tile([1, E], F32)
                nc.vector.tensor_copy(zkd, zkr)
                nc.sync.dma_start(out=zkr_dbg[:, :], in_=zkd)
                for ci in range(16):
                    kned = p_ocp.tile([128, E], F32, name="kned", bufs=1)
                    nc.vector.tensor_copy(kned, kn_e[ci])
                    nc.sync.dma_start(
                        out=kne_dbg.rearrange("(n p) e -> n p e", p=128)[ci],
                        in_=kned)
            invzkr = p_zb.tile([1, E], F32)
            nc.vector.reciprocal(invzkr, zkr)
            invzkr_bf = p_zb.tile([1, E], BF16)
            nc.vector.tensor_copy(invzkr_bf, invzkr)
            repk = p_mm.tile([128, E], F32, name="repk", tag="mm")
            nc.tensor.matmul(repk, onesrb, invzkr_bf, start=True, stop=True)
            repk_sb = p_qk.tile([128, E], BF16)
            nc.vector.tensor_copy(repk_sb, repk)
            if debug:
                rkd = p_ocp.tile([128, E], F32, name="rkd", bufs=1)
                nc.vector.tensor_copy(rkd, repk_sb)
                nc.sync.dma_start(out=repk_dbg[:, :], in_=rkd)
            for ci in range(16):
                nc.vector.tensor_mul(kn_e[ci], kn_e[ci], repk_sb)
            if debug and False:
                for ci in range(16):
                    knd = p_ocp.tile([128, E], F32, name="knd")
                    nc.vector.tensor_copy(knd, kn_e[ci])
                    nc.sync.dma_start(
                        out=kn_dbg.rearrange("(n p) e -> n p e", p=128)[ci],
                        in_=knd)

            # ============ P4: attention map for this core's head ==========
            # Extract this core's head rows from qT_soft/kT_soft with the
            # per-core one-hot selector (exact in bf16), then attn = hq.T@hk.
            hq = p_qk.tile([D, C], BF16)
            hk = p_qk.tile([D, C], BF16)
            for cc in range(CC):
                cs = slice(cc * CW, (cc + 1) * CW)
                hqp = p_mm.tile([D, CW], F32, name="hqp", tag="mm")
                for t in range(4):
                    nc.tensor.matmul(hqp, shead[t], qt_e[t][:, cs],
                                     start=(t == 0), stop=(t == 3))
                nc.vector.tensor_copy(hq[:, cs], hqp)
                hkp = p_mm.tile([D, CW], F32, name="hkp", tag="mm")
                for t in range(4):
                    nc.tensor.matmul(hkp, shead[t], kt_e[t][:, cs],
                                     start=(t == 0), stop=(t == 3))
                nc.vector.tensor_copy(hk[:, cs], hkp)
            for nt in range(16):
                for ms in range(CC):
                    aps_ = p_mm.tile([128, CW], F32, name="aps", tag="mm")
                    nc.tensor.matmul(aps_, hq[:, nt * 128:(nt + 1) * 128],
                                     hk[:, ms * CW:(ms + 1) * CW],
                                     start=True, stop=True)
                    asb = p_acp.tile([128, CW], F32, name="asb")
                    nc.vector.tensor_copy(asb, aps_)
                    nc.sync.dma_start(
                        out=attn[nt * 128:(nt + 1) * 128,
                                 ms * CW:(ms + 1) * CW],
                        in_=asb)

            # ============ P5+P6: context + output =========================
            v3r = v_sp.rearrange("(n p) e -> n p e", p=128)
            o3 = out.rearrange("(n p) e -> n p e", p=128)
            for b in range(RL):
                ctxp = p_ctx.tile([128, E], F32, name="ctxp", tag="ctx")
                for nk in range(16):
                    vsb = p_vload.tile([128, E], BF16, name="vld", tag="vld")
                    nc.sync.dma_start(out=vsb, in_=v3r[b * 16 + nk])
                    # NOTE: start=True clears has_written BITS for the whole
                    # bank, so only the first matmul of the bank may set it
                    # (other quarters then overwrite-on-first-touch since
                    # their bits are still clear).
                    for hp in range(4):
                        nc.tensor.matmul(
                            ctxp[:, hp * 128:(hp + 1) * 128],
                            kn_e[nk][:, hp * 128:(hp + 1) * 128],
                            vsb[:, hp * 128:(hp + 1) * 128],
                            start=(nk == 0 and hp == 0),
                            stop=(nk == 15 and hp == 3))
                if debug:
                    cdb = p_ocp.tile([128, E], F32, name="cdb", bufs=1)
                    nc.vector.tensor_copy(cdb, ctxp)
                    nc.sync.dma_start(
                        out=ctx_dbg.rearrange("(n p) e -> n p e", p=128)[b],
                        in_=cdb)
                # build block-diag ctx tiles (bf16)
                bd = [p_bd.tile([128, 128], BF16, name="bd", tag="bd")
                      for _ in range(4)]
                for hp in range(4):
                    nc.vector.memset(bd[hp], 0.0)
                    nc.vector.tensor_copy(
                        bd[hp][0:64, 0:64],
                        ctxp[0:64, hp * 128:hp * 128 + 64])
                    nc.vector.tensor_copy(
                        bd[hp][64:128, 64:128],
                        ctxp[64:128, hp * 128 + 64:(hp + 1) * 128])
                for nt in range(16):
                    ops_ = p_mm.tile([128, E], F32, name="ops", tag="mm")
                    for hp in range(4):
                        nc.tensor.matmul(
                            ops_[:, hp * 128:(hp + 1) * 128],
                            qt_e[hp][:, nt * 128:(nt + 1) * 128],
                            bd[hp], start=True, stop=True)
                    osb = p_ocp.tile([128, E], F32, name="osb")
                    nc.vector.tensor_copy(osb, ops_)
                    nc.sync.dma_start(out=o3[b * 16 + nt], in_=osb)

    nc.finalize()
    return nc


def _host_prep(x, Wq, bq, Wk, bk, Wv, bv):
    """Build the 8 per-core input maps."""
    import ml_dtypes
    bf16 = ml_dtypes.bfloat16
    x = np.ascontiguousarray(x, dtype=np.float32)
    wqT = np.ascontiguousarray(Wq.T / R, dtype=np.float32)
    wkT = np.ascontiguousarray(Wk.T / R, dtype=np.float32)
    wvT = np.ascontiguousarray(Wv.T, dtype=np.float32)
    ones_r = np.ones((1, 128), dtype=np.float32)
    ones_cb = np.ones((128, 1), dtype=bf16)
    s_sel = np.zeros((E, H), dtype=bf16)
    for h in range(H):
        s_sel[h * D:(h + 1) * D, h] = 1.0
    s_selT = np.ascontiguousarray(s_sel.T)
    ones_rb = np.ones((1, 128), dtype=bf16)
    in_maps = []
    for c in range(N_CORES):
        xs = x[RL * c:RL * (c + 1)]              # [8, 2048, 512]
        xT = np.ascontiguousarray(xs.transpose(2, 0, 1).reshape(E, T))
        s_head = np.zeros((E, D), dtype=bf16)
        s_head[c * D:(c + 1) * D, :] = np.eye(D, dtype=bf16)
        in_maps.append({
            "xT": xT,
            "wqT": wqT, "wkT": wkT, "wvT": wvT,
            "bq_c": bq.reshape(E, 1).astype(np.float32),
            "bk_c": bk.reshape(E, 1).astype(np.float32),
            "bk_r": bk.reshape(1, E).astype(np.float32),
            "bv_r": bv.reshape(1, E).astype(np.float32),
            "ones_r": ones_r,
            "ones_cb": ones_cb,
            "s_sel": s_sel,
            "s_selT": s_selT,
            "ones_rb": ones_rb,
            "s_head": s_head,
        })
    return in_maps


def kernel(x, Wq, bq, Wk, bk, Wv, bv):
    if "nc" not in _CACHE:
        _CACHE["nc"] = build_nc()
    nc = _CACHE["nc"]
    in_maps = _host_prep(np.asarray(x), np.asarray(Wq), np.asarray(bq),
                         np.asarray(Wk), np.asarray(bk),
                         np.asarray(Wv), np.asarray(bv))
    res = run_bass_kernel_spmd(nc, in_maps, list(range(N_CORES)))
    out = np.concatenate(
        [res.results[c]["out"].reshape(RL, C, E) for c in range(N_CORES)], axis=0)
    attn = np.stack([res.results[c]["attn"] for c in range(N_CORES)], axis=0)
    return out, attn
